# revision 35
# baseline (speedup 1.0000x reference)
"""MinusAttention kernel for Trainium2 (8 NeuronCores, Bass/Tile).

Math: score[i,j] = (w.q_i - w.k_j + b) / sqrt(E) with causal mask.
Within a softmax row i, the w.q_i and b terms are constant across j and
cancel, so

    weights[i,j] = g_j / sum_{j'<=i} g_j',   g_j = exp(-w.k_j / sqrt(E))
    out[i,:]     = (sum_{j<=i} g_j V[j,:]) / (sum_{j<=i} g_j)

i.e. a causal cumulative weighted average of V -- O(S*E) per (b,h)
instead of O(L*S*E) -- and the output does not depend on queries at all.

Device kernel per core (4 of the 32 (b,h) pairs), natural layout
[s%128 partitions, (s//128, e) free], per pair:

  - sk[p,k]   = reduce_add_e(ktw[p,k,e])      # DVE; ktw host-prescaled by -w/sqrt(E)
  - g         = exp(sk)                       # ACT  [128,16]
  - wg        = vg * g                        # DVE TT, g broadcast along free;
                                              # vg col 64 is ones -> wg col 64 = g
  - per chunk c (4 blocks): PSUM_c = TriUT @ wg_c   (within-block prefix sums)
  - cw32      = copy(PSUM rows 96:128)        # ACT (PSUM reads must be 32-aligned)
  - bsT[k]    = cw32 row 31 of each block     # tiny SBUF->SBUF DMA
  - rhs_m     = maskT * bsT_bcast             # GPSIMD; maskT[k',k]=1 iff k'<k
  - PSUM_c   += ones16 @ rhs_m_c              # adds carry_k = sum_{k'<k} bs_k'
  - cw        = copy(PSUM)                    # ACT -> SBUF
  - r         = 1/cw[:, :, 64]                # DVE [128,16]
  - out       = cw[:, :, 0:64] * r_bcast      # DVE TT

Pairs are processed in two groups of two with phase-major emission
(wavefront pipelining across engines, dense PE bursts); each pair's kt
streams on the SP HWDGE ring while vg streams on the ACT ring.
"""

import numpy as np

B, L, S, H, E = 4, 2048, 2048, 8, 64
NCORES = 8
PAIRS = (B * H) // NCORES  # (b,h) pairs per core
NBLK = S // 128  # 16
CHUNK = 4  # blocks per PSUM tile: 4*65 = 260 fp32 < 512 (one bank)
NCHUNK = NBLK // CHUNK  # 4
GROUP = 2  # pairs per phase-major group
SCALE = np.float32(1.0 / np.sqrt(np.float32(E)))

TRACE = False
LAST_RESULTS = None

_compiled = None


def _build():
    from concourse import bacc
    import concourse.mybir as mybir
    import concourse.tile as tile
    from concourse.masks import make_upper_triangular
    from concourse.tile_rust import add_dep_helper

    f32 = mybir.dt.float32
    nc = bacc.Bacc("TRN2", target_bir_lowering=False, debug=False)

    ktw = nc.dram_tensor("ktw", [PAIRS, 128, NBLK, E], f32, kind="ExternalInput")
    vg = nc.dram_tensor("vg", [PAIRS, 128, NBLK, E + 1], f32, kind="ExternalInput")
    out = nc.dram_tensor("out", [PAIRS, 128, NBLK, E], f32, kind="ExternalOutput")

    with tile.TileContext(nc) as tc:
        with (
            tc.tile_pool(name="const", bufs=1) as cpool,
            tc.tile_pool(name="ktp", bufs=2 * GROUP) as ktp,
            tc.tile_pool(name="vgp", bufs=2 * GROUP) as vgp,
            tc.tile_pool(name="gp", bufs=2 * GROUP) as gp,
            tc.tile_pool(name="wgp", bufs=2 * GROUP) as wgp,
            tc.tile_pool(name="bsp", bufs=2 * GROUP) as bsp,
            tc.tile_pool(name="rmp", bufs=2 * GROUP * NCHUNK) as rmp,
            tc.tile_pool(name="cwp", bufs=2 * GROUP) as cwp,
            tc.tile_pool(name="rp", bufs=2 * GROUP) as rp,
            tc.tile_pool(name="outp", bufs=2 * GROUP) as outp,
            tc.tile_pool(name="ps", bufs=8, space="PSUM") as psp,
        ):
            tri = cpool.tile([128, 128], f32)
            make_upper_triangular(nc, tri[:], val=1.0, diag=True)
            ones16 = cpool.tile([16, 128], f32)
            nc.gpsimd.memset(ones16[:], 1.0)
            # maskT[k', k, n] = 1 iff k' < k (strictly below target block)
            maskT = cpool.tile([16, NBLK, E + 1], f32)
            nc.gpsimd.memset(maskT[:], 1.0)
            nc.gpsimd.affine_select(
                out=maskT[:],
                in_=maskT[:],
                compare_op=mybir.AluOpType.is_gt,
                fill=0.0,
                base=0,
                # expr = -k' + k > 0  <=>  k' < k
                pattern=[[1, NBLK], [0, E + 1]],
                channel_multiplier=-1,
            )

            prev_wmul = None
            for grp in range(PAIRS // GROUP):
                pairs = list(range(grp * GROUP, (grp + 1) * GROUP))

                kts, vgts = {}, {}
                for p in pairs:
                    kt = ktp.tile([128, NBLK, E], f32, tag="kt")
                    vgt = vgp.tile([128, NBLK, E + 1], f32, tag="vg")
                    if p == 0:
                        # pair 0 gates the whole pipeline: stripe its two
                        # tensors across BOTH HWDGE rings in halves so its
                        # data lands in half the time
                        hb = NBLK // 2
                        nc.sync.dma_start(out=kt[:, 0:hb, :], in_=ktw[p, :, 0:hb, :])
                        nc.scalar.dma_start(out=kt[:, hb:NBLK, :], in_=ktw[p, :, hb:NBLK, :])
                        nc.scalar.dma_start(out=vgt[:, 0:hb, :], in_=vg[p, :, 0:hb, :])
                        nc.sync.dma_start(out=vgt[:, hb:NBLK, :], in_=vg[p, :, hb:NBLK, :])
                    else:
                        # kt on the SP HWDGE ring, vg on the ACT ring: both
                        # of a pair's inputs stream in parallel
                        nc.sync.dma_start(out=kt[:], in_=ktw[p])
                        nc.scalar.dma_start(out=vgt[:], in_=vg[p])
                    kts[p], vgts[p] = kt, vgt

                wgs = {}
                for p in pairs:
                    g = gp.tile([128, NBLK], f32, tag="g")
                    red = nc.vector.tensor_reduce(
                        g[:], kts[p][:], mybir.AxisListType.X, mybir.AluOpType.add
                    )
                    if prev_wmul is not None:
                        # order-only edge: a pair's reduce (gated on its kt
                        # arrival) must not be scheduled ahead of the
                        # previous pair's Wmul in the DVE stream, or the
                        # first matmuls stall on late kt DMAs
                        add_dep_helper(red.ins, prev_wmul.ins, sync=False,
                                       reason="reduce after prev pair wmul")
                    nc.scalar.activation(g[:], g[:], mybir.ActivationFunctionType.Exp)
                    wg = wgp.tile([128, NBLK, E + 1], f32, tag="wg")
                    gb = g[:].to_broadcast([128, NBLK, E + 1])
                    prev_wmul = nc.vector.tensor_tensor(
                        out=wg[:], in0=vgts[p][:], in1=gb, op=mybir.AluOpType.mult
                    )
                    wgs[p] = wg

                pss = {}
                for p in pairs:
                    for c in range(NCHUNK):
                        ps = psp.tile([128, CHUNK, E + 1], f32, tag="ps")
                        nc.tensor.matmul(
                            ps[:], lhsT=tri[:],
                            rhs=wgs[p][:, c * CHUNK : (c + 1) * CHUNK, :],
                            start=True, stop=False, skip_group_check=True,
                        )
                        pss[(p, c)] = ps

                bsTs = {}
                for p in pairs:
                    bsT = bsp.tile([NBLK, 1, E + 1], f32, tag="bs")
                    for c in range(NCHUNK):
                        # block sums live in row 127 of each block's prefix
                        # sums; PSUM reads need 32-aligned bases: copy rows
                        # 96:128 to SBUF, partition-scatter row 31 via DMA
                        c32 = cwp.tile([32, CHUNK, E + 1], f32, tag="cw32")
                        nc.scalar.copy(c32[:], pss[(p, c)][96:128, :, :])
                        nc.sync.dma_start(
                            out=bsT[c * CHUNK : (c + 1) * CHUNK, :, :],
                            in_=c32[31:32, :, :],
                        )
                    bsTs[p] = bsT

                rms = {}
                for p in pairs:
                    chunks = []
                    for c in range(NCHUNK):
                        rm = rmp.tile([16, CHUNK, E + 1], f32, tag="rm")
                        # chunk c's carries only involve block sums k' < 4c+4,
                        # i.e. rows already delivered by bs chunks 0..c
                        nc.gpsimd.tensor_tensor(
                            out=rm[:],
                            in0=maskT[:, c * CHUNK : (c + 1) * CHUNK, :],
                            in1=bsTs[p][:].broadcast_to([NBLK, CHUNK, E + 1]),
                            op=mybir.AluOpType.mult,
                        )
                        chunks.append(rm)
                    rms[p] = chunks

                for p in pairs:
                    for c in range(NCHUNK):
                        nc.tensor.matmul(
                            pss[(p, c)][:], lhsT=ones16[:],
                            rhs=rms[p][c][:],
                            start=False, stop=True, skip_group_check=True,
                        )

                cws = {}
                for p in pairs:
                    cw = cwp.tile([128, NBLK, E + 1], f32, tag="cw")
                    for c in range(NCHUNK):
                        # PSUM drain on DVE: runs in parallel with ACT's c32
                        # copies of the next group's block-sum extraction
                        nc.vector.tensor_copy(
                            cw[:, c * CHUNK : (c + 1) * CHUNK, :], pss[(p, c)][:]
                        )
                    cws[p] = cw

                for p in pairs:
                    r = rp.tile([128, NBLK], f32, tag="r")
                    nc.vector.reciprocal(
                        r[:], cws[p][:, :, E : E + 1].rearrange("p k o -> p (k o)")
                    )
                    ot = outp.tile([128, NBLK, E], f32, tag="out")
                    rb = r[:].to_broadcast([128, NBLK, E])
                    nc.vector.tensor_tensor(
                        out=ot[:], in0=cws[p][:, :, 0:E], in1=rb, op=mybir.AluOpType.mult
                    )
                    nc.sync.dma_start(out=out[p], in_=ot[:])

    nc.compile()
    return nc


def _get_compiled():
    global _compiled
    if _compiled is None:
        _compiled = _build()
    return _compiled


def prep_inputs(keys: np.ndarray, values: np.ndarray, w_score: np.ndarray):
    """Host-side reshard: returns in_maps (list of 8 dicts)."""
    keys = np.asarray(keys, dtype=np.float32)
    values = np.asarray(values, dtype=np.float32)
    w = np.asarray(w_score, dtype=np.float32)

    # [B,S,H,E] -> [B,H,S,E] -> [B*H, NBLK, 128, E] -> [B*H, 128, NBLK, E]
    kt = keys.transpose(0, 2, 1, 3).reshape(B * H, NBLK, 128, E)
    kt = (kt * (-SCALE * w)).transpose(0, 2, 1, 3)

    v = values.transpose(0, 2, 1, 3).reshape(B * H, NBLK, 128, E)
    vg = np.concatenate([v, np.ones((B * H, NBLK, 128, 1), np.float32)], axis=-1)
    vg = vg.transpose(0, 2, 1, 3)  # [B*H, 128, NBLK, E+1]

    in_maps = []
    for c in range(NCORES):
        sl = slice(PAIRS * c, PAIRS * (c + 1))
        in_maps.append(
            {
                "ktw": np.ascontiguousarray(kt[sl]),
                "vg": np.ascontiguousarray(vg[sl]),
            }
        )
    return in_maps


def assemble_output(results) -> np.ndarray:
    # results[c]["out"]: [PAIRS, 128, NBLK, E]; s = 128*k + partition
    arr = np.stack([np.asarray(r["out"]) for r in results])  # [8, PAIRS, 128, NBLK, E]
    arr = arr.reshape(B * H, 128, NBLK, E).transpose(0, 2, 1, 3)  # [B*H, NBLK, 128, E]
    arr = arr.reshape(B, H, L, E).transpose(0, 2, 1, 3)  # [B, L, H, E]
    return np.ascontiguousarray(arr)


def kernel(queries=None, keys=None, values=None, w_score=None, b_score=None, attn_mask=None, **_):
    global LAST_RESULTS
    from concourse.bass_utils import run_bass_kernel_spmd

    nc = _get_compiled()
    in_maps = prep_inputs(keys, values, w_score)
    res = run_bass_kernel_spmd(nc, in_maps, core_ids=list(range(NCORES)), trace=TRACE)
    LAST_RESULTS = res
    return assemble_output(res.results)


# revision 36
# speedup vs baseline: 1.0999x; 1.0999x over previous
"""MinusAttention kernel for Trainium2 (8 NeuronCores, Bass/Tile).

Math: score[i,j] = (w.q_i - w.k_j + b) / sqrt(E) with causal mask.
Within a softmax row i, the w.q_i and b terms are constant across j and
cancel, so

    weights[i,j] = g_j / sum_{j'<=i} g_j',   g_j = exp(-w.k_j / sqrt(E))
    out[i,:]     = (sum_{j<=i} g_j V[j,:]) / (sum_{j<=i} g_j)

i.e. a causal cumulative weighted average of V -- O(S*E) per (b,h)
instead of O(L*S*E) -- and the output does not depend on queries at all.

Device kernel per core (4 of the 32 (b,h) pairs), natural layout
[s%128 partitions, (s//128, e) free], per pair:

  - sk[p,k]   = reduce_add_e(ktw[p,k,e])      # DVE; ktw host-prescaled by -w/sqrt(E)
  - g         = exp(sk)                       # ACT  [128,16]
  - wg        = vg * g                        # DVE TT, g broadcast along free;
                                              # vg col 64 is ones -> wg col 64 = g
  - per chunk c (4 blocks): PSUM_c = TriUT @ wg_c   (within-block prefix sums)
  - cw32      = copy(PSUM rows 96:128)        # ACT (PSUM reads must be 32-aligned)
  - bsT[k]    = cw32 row 31 of each block     # tiny SBUF->SBUF DMA
  - rhs_m     = maskT * bsT_bcast             # GPSIMD; maskT[k',k]=1 iff k'<k
  - PSUM_c   += ones16 @ rhs_m_c              # adds carry_k = sum_{k'<k} bs_k'
  - cw        = copy(PSUM)                    # ACT -> SBUF
  - r         = 1/cw[:, :, 64]                # DVE [128,16]
  - out       = cw[:, :, 0:64] * r_bcast      # DVE TT

Pairs are processed in two groups of two with phase-major emission
(wavefront pipelining across engines, dense PE bursts); each pair's kt
streams on the SP HWDGE ring while vg streams on the ACT ring.
"""

import numpy as np

B, L, S, H, E = 4, 2048, 2048, 8, 64
NCORES = 8
PAIRS = (B * H) // NCORES  # (b,h) pairs per core
NBLK = S // 128  # 16
CHUNK = 4  # blocks per PSUM tile: 4*65 = 260 fp32 < 512 (one bank)
NCHUNK = NBLK // CHUNK  # 4
GROUP = 2  # pairs per phase-major group
SCALE = np.float32(1.0 / np.sqrt(np.float32(E)))

TRACE = False
LAST_RESULTS = None

_compiled = None


def _build():
    from concourse import bacc
    import concourse.mybir as mybir
    import concourse.tile as tile
    from concourse.masks import make_upper_triangular
    from concourse.tile_rust import add_dep_helper

    f32 = mybir.dt.float32
    nc = bacc.Bacc("TRN2", target_bir_lowering=False, debug=False)

    ktw = nc.dram_tensor("ktw", [PAIRS, 128, NBLK, E], f32, kind="ExternalInput")
    vg = nc.dram_tensor("vg", [PAIRS, 128, NBLK, E + 1], f32, kind="ExternalInput")
    out = nc.dram_tensor("out", [PAIRS, 128, NBLK, E], f32, kind="ExternalOutput")

    with tile.TileContext(nc) as tc:
        with (
            tc.tile_pool(name="const", bufs=1) as cpool,
            tc.tile_pool(name="ktp", bufs=2 * GROUP) as ktp,
            tc.tile_pool(name="vgp", bufs=2 * GROUP) as vgp,
            tc.tile_pool(name="gp", bufs=2 * GROUP) as gp,
            tc.tile_pool(name="wgp", bufs=2 * GROUP) as wgp,
            tc.tile_pool(name="bsp", bufs=2 * GROUP) as bsp,
            tc.tile_pool(name="rmp", bufs=2 * GROUP * NCHUNK) as rmp,
            tc.tile_pool(name="cwp", bufs=2 * GROUP) as cwp,
            tc.tile_pool(name="rp", bufs=2 * GROUP) as rp,
            tc.tile_pool(name="outp", bufs=2 * GROUP) as outp,
            tc.tile_pool(name="ps", bufs=8, space="PSUM") as psp,
        ):
            tri = cpool.tile([128, 128], f32)
            make_upper_triangular(nc, tri[:], val=1.0, diag=True)
            ones16 = cpool.tile([16, 128], f32)
            nc.gpsimd.memset(ones16[:], 1.0)
            # maskT[k', k, n] = 1 iff k' < k (strictly below target block)
            maskT = cpool.tile([16, NBLK, E + 1], f32)
            nc.gpsimd.memset(maskT[:], 1.0)
            nc.gpsimd.affine_select(
                out=maskT[:],
                in_=maskT[:],
                compare_op=mybir.AluOpType.is_gt,
                fill=0.0,
                base=0,
                # expr = -k' + k > 0  <=>  k' < k
                pattern=[[1, NBLK], [0, E + 1]],
                channel_multiplier=-1,
            )

            prev_wmul = None
            for grp in range(PAIRS // GROUP):
                pairs = list(range(grp * GROUP, (grp + 1) * GROUP))

                kts, vgts = {}, {}
                for p in pairs:
                    kt = ktp.tile([128, NBLK, E], f32, tag="kt")
                    vgt = vgp.tile([128, NBLK, E + 1], f32, tag="vg")
                    # kt on the SP HWDGE ring, vg on the ACT ring: both of a
                    # pair's inputs stream in parallel, earlier pairs first
                    nc.sync.dma_start(out=kt[:], in_=ktw[p])
                    nc.scalar.dma_start(out=vgt[:], in_=vg[p])
                    kts[p], vgts[p] = kt, vgt

                wgs = {}
                for p in pairs:
                    g = gp.tile([128, NBLK], f32, tag="g")
                    red = nc.vector.tensor_reduce(
                        g[:], kts[p][:], mybir.AxisListType.X, mybir.AluOpType.add
                    )
                    if prev_wmul is not None:
                        # order-only edge: a pair's reduce (gated on its kt
                        # arrival) must not be scheduled ahead of the
                        # previous pair's Wmul in the DVE stream, or the
                        # first matmuls stall on late kt DMAs
                        add_dep_helper(red.ins, prev_wmul.ins, sync=False,
                                       reason="reduce after prev pair wmul")
                    nc.scalar.activation(g[:], g[:], mybir.ActivationFunctionType.Exp)
                    wg = wgp.tile([128, NBLK, E + 1], f32, tag="wg")
                    gb = g[:].to_broadcast([128, NBLK, E + 1])
                    prev_wmul = nc.vector.tensor_tensor(
                        out=wg[:], in0=vgts[p][:], in1=gb, op=mybir.AluOpType.mult
                    )
                    wgs[p] = wg

                pss = {}
                for p in pairs:
                    for c in range(NCHUNK):
                        ps = psp.tile([128, CHUNK, E + 1], f32, tag="ps")
                        nc.tensor.matmul(
                            ps[:], lhsT=tri[:],
                            rhs=wgs[p][:, c * CHUNK : (c + 1) * CHUNK, :],
                            start=True, stop=False, skip_group_check=True,
                        )
                        pss[(p, c)] = ps

                bsTs = {}
                for p in pairs:
                    bsT = bsp.tile([NBLK, 1, E + 1], f32, tag="bs")
                    for c in range(NCHUNK):
                        # block sums live in row 127 of each block's prefix
                        # sums; PSUM reads need 32-aligned bases: copy rows
                        # 96:128 to SBUF, partition-scatter row 31 via DMA
                        c32 = cwp.tile([32, CHUNK, E + 1], f32, tag="cw32")
                        nc.scalar.copy(c32[:], pss[(p, c)][96:128, :, :])
                        nc.sync.dma_start(
                            out=bsT[c * CHUNK : (c + 1) * CHUNK, :, :],
                            in_=c32[31:32, :, :],
                        )
                    bsTs[p] = bsT

                rms = {}
                for p in pairs:
                    chunks = []
                    for c in range(NCHUNK):
                        rm = rmp.tile([16, CHUNK, E + 1], f32, tag="rm")
                        # chunk c's carries only involve block sums k' < 4c+4,
                        # i.e. rows already delivered by bs chunks 0..c
                        nc.gpsimd.tensor_tensor(
                            out=rm[:],
                            in0=maskT[:, c * CHUNK : (c + 1) * CHUNK, :],
                            in1=bsTs[p][:].broadcast_to([NBLK, CHUNK, E + 1]),
                            op=mybir.AluOpType.mult,
                        )
                        chunks.append(rm)
                    rms[p] = chunks

                for p in pairs:
                    for c in range(NCHUNK):
                        nc.tensor.matmul(
                            pss[(p, c)][:], lhsT=ones16[:],
                            rhs=rms[p][c][:],
                            start=False, stop=True, skip_group_check=True,
                        )

                cws = {}
                for p in pairs:
                    cw = cwp.tile([128, NBLK, E + 1], f32, tag="cw")
                    for c in range(NCHUNK):
                        # PSUM drain on DVE: runs in parallel with ACT's c32
                        # copies of the next group's block-sum extraction
                        nc.vector.tensor_copy(
                            cw[:, c * CHUNK : (c + 1) * CHUNK, :], pss[(p, c)][:]
                        )
                    cws[p] = cw

                for p in pairs:
                    r = rp.tile([128, NBLK], f32, tag="r")
                    nc.vector.reciprocal(
                        r[:], cws[p][:, :, E : E + 1].rearrange("p k o -> p (k o)")
                    )
                    ot = outp.tile([128, NBLK, E], f32, tag="out")
                    rb = r[:].to_broadcast([128, NBLK, E])
                    nc.vector.tensor_tensor(
                        out=ot[:], in0=cws[p][:, :, 0:E], in1=rb, op=mybir.AluOpType.mult
                    )
                    nc.sync.dma_start(out=out[p], in_=ot[:])

    nc.compile()
    return nc


def _get_compiled():
    global _compiled
    if _compiled is None:
        _compiled = _build()
    return _compiled


def prep_inputs(keys: np.ndarray, values: np.ndarray, w_score: np.ndarray):
    """Host-side reshard: returns in_maps (list of 8 dicts)."""
    keys = np.asarray(keys, dtype=np.float32)
    values = np.asarray(values, dtype=np.float32)
    w = np.asarray(w_score, dtype=np.float32)

    # [B,S,H,E] -> [B,H,S,E] -> [B*H, NBLK, 128, E] -> [B*H, 128, NBLK, E]
    kt = keys.transpose(0, 2, 1, 3).reshape(B * H, NBLK, 128, E)
    kt = (kt * (-SCALE * w)).transpose(0, 2, 1, 3)

    v = values.transpose(0, 2, 1, 3).reshape(B * H, NBLK, 128, E)
    vg = np.concatenate([v, np.ones((B * H, NBLK, 128, 1), np.float32)], axis=-1)
    vg = vg.transpose(0, 2, 1, 3)  # [B*H, 128, NBLK, E+1]

    in_maps = []
    for c in range(NCORES):
        sl = slice(PAIRS * c, PAIRS * (c + 1))
        in_maps.append(
            {
                "ktw": np.ascontiguousarray(kt[sl]),
                "vg": np.ascontiguousarray(vg[sl]),
            }
        )
    return in_maps


def assemble_output(results) -> np.ndarray:
    # results[c]["out"]: [PAIRS, 128, NBLK, E]; s = 128*k + partition
    arr = np.stack([np.asarray(r["out"]) for r in results])  # [8, PAIRS, 128, NBLK, E]
    arr = arr.reshape(B * H, 128, NBLK, E).transpose(0, 2, 1, 3)  # [B*H, NBLK, 128, E]
    arr = arr.reshape(B, H, L, E).transpose(0, 2, 1, 3)  # [B, L, H, E]
    return np.ascontiguousarray(arr)


def kernel(queries=None, keys=None, values=None, w_score=None, b_score=None, attn_mask=None, **_):
    global LAST_RESULTS
    from concourse.bass_utils import run_bass_kernel_spmd

    nc = _get_compiled()
    in_maps = prep_inputs(keys, values, w_score)
    res = run_bass_kernel_spmd(nc, in_maps, core_ids=list(range(NCORES)), trace=TRACE)
    LAST_RESULTS = res
    return assemble_output(res.results)
